# revision 19
# baseline (speedup 1.0000x reference)
"""Multi-head self-attention (RoPE, causal) Trainium2 Bass kernel, 8 NeuronCores.

Sharding: data-parallel over batch (B=2) x tensor-parallel over heads
(16 heads -> 4 groups of 4). Core c handles batch b=c//4, heads 4*(c%4)..4*(c%4)+3.
Each core computes its 4 heads' attention plus a partial output projection;
the host sums the 4 partial outputs per batch element (bf16 partials).

Single fused pipeline (v2): attention kt-iterations (both head pairs
interleaved per kt) are the ACT-bound backbone; projection and output-
projection matmul units are emitted between kt-iterations as PE filler so
the tensor engine stays busy while ACT runs exp. All PSUM flows through two
pools of [128, 1024] f32 tiles (2 banks each, bufs=2): "big" for scores /
QK-proj / V-proj / out-proj tiles, "po" for the two per-pair output
accumulators (V rows + ones row -> denominators ride along in row 64).
Warmup matmuls un-throttle the PE HAM clock gate before real work arrives;
input DMAs are column-split and priority-ordered so the first projection
starts ~4us in. Softmax normalize uses gpsimd partition_broadcast (no DRAM
round trip). Output partials stored bf16.
"""
import sys, math

sys.path.insert(0, "/opt/trn_rl_repo")

import numpy as np
import ml_dtypes

import concourse.bacc as bacc
import concourse.bass as bass
import concourse.mybir as mybir
import concourse.tile as tile
from concourse.bass_utils import run_bass_kernel_spmd

BF16 = mybir.dt.bfloat16
F32 = mybir.dt.float32
NPBF16 = ml_dtypes.bfloat16

D_MODEL = 1024
D_HEAD = 64
HALF = D_HEAD // 2
ROPE_THETA = 10000.0
N_CORES = 8
C = 256  # channels per core (4 heads x 64)
SWAP32 = [i ^ 1 for i in range(32)]


def _body(nc, tc, L, pp, rtp, ptp, rip, osp, bigp, pop):
    n_lt = L // 128
    qw = 512
    n_qch = L // qw
    scale = 1.0 / math.sqrt(D_HEAD)

    xt_d = nc.dram_tensor("xt", [D_MODEL, L], BF16, kind="ExternalInput").ap()
    wq_d = nc.dram_tensor("wqt", [D_MODEL, C], BF16, kind="ExternalInput").ap()
    wk_d = nc.dram_tensor("wkt", [D_MODEL, C], BF16, kind="ExternalInput").ap()
    wv_d = nc.dram_tensor("wvt", [D_MODEL, C], BF16, kind="ExternalInput").ap()
    wo_d = nc.dram_tensor("wot", [C, D_MODEL], BF16, kind="ExternalInput").ap()
    cos_d = nc.dram_tensor("cosb", [128, L], BF16, kind="ExternalInput").ap()
    sin_d = nc.dram_tensor("ssin", [128, L], BF16, kind="ExternalInput").ap()
    id_d = nc.dram_tensor("ident", [128, 128], BF16, kind="ExternalInput").ap()
    ng_d = nc.dram_tensor("negt", [128, 128], BF16, kind="ExternalInput").ap()
    out_d = nc.dram_tensor("out", [L, D_MODEL], BF16, kind="ExternalOutput").ap()

    # ---- persistent SBUF tensors
    wq = pp.tile([128, 8, C], BF16)
    wk = pp.tile([128, 8, C], BF16)
    wv = pp.tile([128, 8, C], BF16)
    wo = pp.tile([128, 2, D_MODEL], BF16)
    cs = pp.tile([128, L], BF16)
    sn = pp.tile([128, L], BF16)
    ident = pp.tile([128, 128], BF16)
    negt = pp.tile([128, 128], BF16)
    warm = pp.tile([128, 512], BF16)
    qt_c = [pp.tile([128, 2, qw], BF16, name=f"qt{i}") for i in range(n_qch)]
    kt_c = [pp.tile([128, 2, qw], BF16, name=f"ktc{i}") for i in range(n_qch)]
    # per l-tile, per head: [64 V-channels | 64 ones]; the ones columns make
    # PV deposit the softmax denominator on PSUM rows 64..127 (partition-
    # aligned with the 64 A' rows) so normalize needs no broadcast.
    vt_c = [pp.tile([128, 4, 512], BF16, name=f"vt{i}") for i in range(n_qch)]
    at = pp.tile([128, 2, L], BF16)
    xts = [pp.tile([128, L], BF16, name=f"xt{i}") for i in range(8)]

    # ---- warmup + memsets (no DMA dependency; un-throttle the PE HAM gate)
    nc.gpsimd.memset(warm[:], 0.0)
    for i in range(n_qch):
        ov = vt_c[i][:, :, :].rearrange("p l (h x) -> p l h x", x=128)
        nc.gpsimd.memset(ov[:, :, :, 64:128], 1.0)
    wps = bigp.tile([128, 1024], F32, tag="big", name="warm_ps")
    for r in range(10):
        nc.tensor.matmul(wps[:, 0:512], lhsT=warm[:, 0:128], rhs=warm[:],
                         start=(r == 0), stop=(r == 9), skip_group_check=True)

    # ---- input DMAs. SP queue carries weights + x in need order; the ACT
    # HWDGE queue carries the small RoPE/mask tables so they arrive early
    # without serializing behind the big transfers.
    nc.sync.dma_start(out=wq[:], in_=wq_d.rearrange("(a p) c -> p a c", p=128))
    for i in range(8):
        nc.sync.dma_start(out=xts[i][:, 0:512], in_=xt_d[i * 128:(i + 1) * 128, 0:512])
    nc.sync.dma_start(out=wk[:], in_=wk_d.rearrange("(a p) c -> p a c", p=128))
    nc.sync.dma_start(out=wv[:], in_=wv_d.rearrange("(a p) c -> p a c", p=128))
    for qc in range(1, n_qch):
        ls = qc * qw
        for i in range(8):
            nc.sync.dma_start(out=xts[i][:, ls:ls + qw],
                              in_=xt_d[i * 128:(i + 1) * 128, ls:ls + qw])
    nc.sync.dma_start(out=wo[:], in_=wo_d.rearrange("(a p) e -> p a e", p=128))
    nc.scalar.dma_start(out=cs[:, 0:512], in_=cos_d[:, 0:512])
    nc.scalar.dma_start(out=sn[:, 0:512], in_=sin_d[:, 0:512])
    nc.scalar.dma_start(out=ident[:], in_=id_d)
    nc.scalar.dma_start(out=negt[:], in_=ng_d)
    nc.scalar.dma_start(out=cs[:, 512:L], in_=cos_d[:, 512:L])
    nc.scalar.dma_start(out=sn[:, 512:L], in_=sin_d[:, 512:L])

    # ---- emission units -----------------------------------------------------
    def qk_unit(nm, w, dstc, qc):
        """Project Q^T or K^T for chunk qc (16 matmuls) + RoPE into dstc[qc]."""
        ls = qc * qw
        p = bigp.tile([128, 2, qw], F32, tag="big", name=f"ps_{nm}_{qc}")
        for ct in (0, 1):
            for dt_ in range(8):
                nc.tensor.matmul(
                    p[:, ct, :],
                    lhsT=w[:, dt_, ct * 128:ct * 128 + 128],
                    rhs=xts[dt_][:, ls:ls + qw],
                    start=(dt_ == 0), stop=(dt_ == 7), skip_group_check=True)
        dst = dstc[qc]
        for ct in (0, 1):
            sh = rtp.tile([128, qw], F32, tag="t", name=f"sh_{nm}{ct}{qc}")
            t1 = rtp.tile([128, qw], F32, tag="t", name=f"t1_{nm}{ct}{qc}")
            t2 = rtp.tile([128, qw], F32, tag="t", name=f"t2_{nm}{ct}{qc}")
            nc.vector.stream_shuffle(sh[:], p[:, ct, :], SWAP32)
            nc.vector.tensor_mul(t1[:], p[:, ct, :], cs[:, ls:ls + qw])
            nc.gpsimd.tensor_mul(t2[:], sh[:], sn[:, ls:ls + qw])
            nc.gpsimd.tensor_add(dst[:, ct, :], t1[:], t2[:])

    def v_unit(qc):
        """Project V for chunk qc's 4 l-tiles (16 matmuls) + copy to vt_c."""
        pv = bigp.tile([128, 4, C], F32, tag="big", name=f"pv_{qc}")
        for lt in range(4):
            lg = qc * 4 + lt
            for dt_ in range(8):
                nc.tensor.matmul(
                    pv[:, lt, :],
                    lhsT=xts[dt_][:, lg * 128:lg * 128 + 128],
                    rhs=wv[:, dt_, :],
                    start=(dt_ == 0), stop=(dt_ == 7), skip_group_check=True)
        ov = vt_c[qc][:, :, :].rearrange("p l (h x) -> p l h x", x=128)[:, :, :, 0:64]
        nc.vector.tensor_copy(ov, pv[:].rearrange("p l (h x) -> p l h x", x=64))

    def outproj_unit(qc, lt):
        """Output projection for l-tile qc*4+lt (4 matmuls) + store bf16."""
        lg = qc * 4 + lt
        pout = bigp.tile([128, 1024], F32, tag="big", name=f"po_{lg}")
        for ct in (0, 1):
            for eh in (0, 1):
                nc.tensor.matmul(
                    pout[:, eh * 512:eh * 512 + 512],
                    lhsT=at[:, ct, lg * 128:lg * 128 + 128],
                    rhs=wo[:, ct, eh * 512:eh * 512 + 512],
                    start=(ct == 0), stop=(ct == 1), skip_group_check=True)
        stg = osp.tile([128, 1024], BF16, tag="stg", name=f"stg_{lg}")
        nc.vector.tensor_copy(stg[:, 0:512], pout[:, 0:512])
        nc.scalar.copy(stg[:, 512:1024], pout[:, 512:1024])
        nc.sync.dma_start(out=out_d[lg * 128:lg * 128 + 128, :], in_=stg[:])

    def normalize(qc, pair, po):
        """Softmax-normalize po -> at[:, pair, chunk qc] (bf16).

        One f32 copy drains po (PSUM freed for the next chunk's PV); the
        divide then runs off the PSUM critical path. DVE ops must be
        partition-aligned, so the denominator row is gpsimd-broadcast to
        partitions 0..63 before the aligned reciprocal + multiply."""
        qs = qc * qw
        rrow = rip.tile([1, 1024], F32, tag="rrow", name=f"rr_{pair}_{qc}")
        nc.vector.tensor_copy(rrow[:], po[64:65, :])
        posb = rip.tile([64, 1024], F32, tag="posb", name=f"posb_{pair}_{qc}")
        nc.vector.tensor_copy(posb[:], po[0:64, :])
        pb0 = rip.tile([64, 1024], F32, tag="pb0", name=f"pb0_{pair}_{qc}")
        nc.gpsimd.partition_broadcast(pb0[:], rrow[:])
        pbi = rip.tile([64, 1024], F32, tag="pbi", name=f"pbi_{pair}_{qc}")
        nc.vector.reciprocal_approx_fast(out=pbi[:], in_=pb0[:])
        tm = rip.tile([64, 1024], BF16, tag="tm", name=f"tm_{pair}_{qc}")
        nc.vector.tensor_mul(tm[:], posb[:], pbi[:])
        nc.vector.tensor_copy(at[0:64, pair, qs:qs + qw], tm[:, 0:qw])
        nc.sync.dma_start(out=at[64:128, pair, qs:qs + qw],
                          in_=tm[:, 512:512 + qw])

    # ---- fused pipeline -----------------------------------------------------
    filler = []
    qk_unit("q", wq, qt_c, 0)
    qk_unit("k", wk, kt_c, 0)
    v_unit(0)
    for qc in range(1, n_qch):
        filler.append((lambda qc=qc: qk_unit("q", wq, qt_c, qc)))
        filler.append((lambda qc=qc: qk_unit("k", wk, kt_c, qc)))
        filler.append((lambda qc=qc: v_unit(qc)))

    # filler units that MUST be emitted before att(qc) starts: proj(qc) is
    # the first 3*qc entries of the original list; track consumed count.
    consumed = [0]

    def emit_filler(n):
        for _ in range(n):
            if filler:
                filler.pop(0)()
                consumed[0] += 1

    nfill = 0
    for qc in range(n_qch):
        qs = qc * qw
        ktmax = (qs + qw) // 128
        po = {}
        for pair in range(2):
            po[pair] = pop.tile([128, 1024], F32, tag="po",
                                name=f"acc_{pair}_{qc}")
        for kt in range(ktmax):
            off = kt * 128 - qs
            qlo = max(0, off)
            kc, ko = kt // 4, (kt % 4) * 128
            pts = {}
            for pair in range(2):
                pt_ps = bigp.tile([128, 1024], F32, tag="big",
                                  name=f"pt_{pair}_{qc}_{kt}")
                for hloc in range(2):
                    nc.tensor.matmul(
                        pt_ps[:, 512 * hloc + qlo:512 * hloc + qw],
                        lhsT=kt_c[kc][64 * hloc:64 * hloc + 64, pair,
                                      ko:ko + 128],
                        rhs=qt_c[qc][64 * hloc:64 * hloc + 64, pair,
                                     qlo:qw],
                        start=True, stop=(off < 0),
                        tile_position=(64 * hloc, 0),
                        skip_group_check=True)
                    if off >= 0:
                        # causal mask: accumulate -240*[q<k] onto the
                        # 128-wide diagonal block; exp then yields ~0 there
                        nc.tensor.matmul(
                            pt_ps[:, 512 * hloc + qlo:512 * hloc + qlo + 128],
                            lhsT=ident[:],
                            rhs=negt[:],
                            start=False, stop=True,
                            skip_group_check=True)
                pts[pair] = pt_ps
            ptsb = {}
            for pair in range(2):
                pt_sb = ptp.tile([128, 1024], BF16, tag="p",
                                 name=f"ptsb_{pair}_{qc}_{kt}")
                pv_ps = pts[pair][:, :].rearrange("p (h x) -> p h x", h=2)
                pv_sb = pt_sb[:, :].rearrange("p (h x) -> p h x", h=2)
                nc.scalar.activation(pv_sb[:, :, qlo:qw], pv_ps[:, :, qlo:qw],
                                     mybir.ActivationFunctionType.Exp,
                                     scale=scale)
                ptsb[pair] = pt_sb
            for pair in range(2):
                for hloc in range(2):
                    h = 2 * pair + hloc
                    nc.tensor.matmul(
                        po[pair][:, 512 * hloc + qlo:512 * hloc + qw],
                        lhsT=vt_c[kc][:, kt % 4, 128 * h:128 * h + 128],
                        rhs=ptsb[pair][:, 512 * hloc + qlo:512 * hloc + qw],
                        start=(kt == 0), stop=(kt == ktmax - 1),
                        skip_group_check=True)
            # pacing: one filler unit every other kt keeps PE fed while ACT
            # runs exp; drain chunk qc+1's projections before its att starts
            nfill += 1
            if nfill % 2 == 0:
                emit_filler(1)
        if qc + 1 < n_qch:
            drain = 3 * (qc + 1) - consumed[0]
            emit_filler(max(0, drain))
        for pair in range(2):
            normalize(qc, pair, po[pair])
        for lt in range(4):
            filler.append((lambda qc=qc, lt=lt: outproj_unit(qc, lt)))
    emit_filler(len(filler))


def build_nc(L=2048):
    """Build + compile the per-core Bass program (same NEFF on all 8 cores)."""
    assert L % 512 == 0
    nc = bacc.Bacc("TRN2", target_bir_lowering=False, debug=False,
                   num_devices=N_CORES)
    with tile.TileContext(nc) as tc:
        with tc.tile_pool(name="persist", bufs=1) as pp, \
             tc.tile_pool(name="ropet", bufs=6) as rtp, \
             tc.tile_pool(name="ptp", bufs=3) as ptp, \
             tc.tile_pool(name="rinvp", bufs=2) as rip, \
             tc.tile_pool(name="ostg", bufs=3) as osp, \
             tc.tile_pool(name="bigps", bufs=2, space="PSUM") as bigp, \
             tc.tile_pool(name="pops", bufs=2, space="PSUM") as pop:
            _body(nc, tc, L, pp, rtp, ptp, rip, osp, bigp, pop)
    nc.compile()
    return nc


_NC_CACHE = {}


def _get_nc(L):
    if L not in _NC_CACHE:
        _NC_CACHE[L] = build_nc(L)
    return _NC_CACHE[L]


def make_inputs(x, token_positions, Wq, Wk, Wv, Wo):
    """Host-side shard/layout prep -> list of 8 per-core input dicts."""
    B, L, _ = x.shape
    pos = np.asarray(token_positions).astype(np.float64)
    S = ROPE_THETA ** (-2.0 / D_HEAD)
    thetas = S ** np.arange(HALF, dtype=np.float64)
    ang = pos[:, None] * thetas[None, :]          # [L, 32]
    cosL = np.cos(ang).T                          # [32, L]
    sinL = np.sin(ang).T
    # per-channel tables on the natural (head, dim) layout:
    # row p (within a 64-row head block): pair i = (p%64)//2
    # cosb[p] = cos(theta_i * pos); ssin[p] = -sin if dim even else +sin
    cosb = np.empty((128, L), dtype=np.float64)
    ssin = np.empty((128, L), dtype=np.float64)
    for p in range(128):
        i = (p % 64) // 2
        cosb[p] = cosL[i]
        ssin[p] = -sinL[i] if (p % 2 == 0) else sinL[i]
    cosb = cosb.astype(NPBF16)
    ssin = ssin.astype(NPBF16)

    r = np.arange(128)[:, None]
    col = np.arange(128)[None, :]
    ident = np.eye(128, dtype=NPBF16)
    negt = np.where(col < r, -240.0, 0.0).astype(NPBF16)  # pre-exp mask bias

    xts = [np.ascontiguousarray(x[b].astype(NPBF16).T) for b in range(B)]
    in_maps = []
    shard_cache = {}
    for core in range(N_CORES):
        b, hg = core // 4, core % 4
        if hg not in shard_cache:
            rows = slice(hg * 256, hg * 256 + 256)
            shard_cache[hg] = {
                "wqt": np.ascontiguousarray(Wq[rows].astype(NPBF16).T),
                "wkt": np.ascontiguousarray(Wk[rows].astype(NPBF16).T),
                "wvt": np.ascontiguousarray(Wv[rows].astype(NPBF16).T),
                "wot": np.ascontiguousarray(Wo[:, rows].astype(NPBF16).T),
            }
        m = dict(shard_cache[hg])
        m["xt"] = xts[b]
        m["cosb"] = cosb
        m["ssin"] = ssin
        m["ident"] = ident
        m["negt"] = negt
        in_maps.append(m)
    return in_maps


def kernel(x, token_positions, Wq, Wk, Wv, Wo):
    x = np.asarray(x); Wq = np.asarray(Wq); Wk = np.asarray(Wk)
    Wv = np.asarray(Wv); Wo = np.asarray(Wo)
    B, L, _ = x.shape
    nc = _get_nc(L)
    in_maps = make_inputs(x, token_positions, Wq, Wk, Wv, Wo)
    res = run_bass_kernel_spmd(nc, in_maps, core_ids=list(range(N_CORES)))
    out = np.zeros((B, L, D_MODEL), dtype=np.float32)
    for core in range(N_CORES):
        out[core // 4] += res.results[core]["out"].astype(np.float32)
    return out


# revision 34
# speedup vs baseline: 1.2060x; 1.2060x over previous
"""Multi-head self-attention (RoPE, causal) Trainium2 Bass kernel, 8 NeuronCores.

Sharding: data-parallel over batch (B=2) x tensor-parallel over heads
(16 heads -> 4 groups of 4). Core c handles batch b=c//4, heads 4*(c%4)..4*(c%4)+3.
Each core computes its 4 heads' attention plus a partial output projection;
the host sums the 4 partial outputs per batch element (bf16 partials).

Single fused pipeline (v2): attention kt-iterations (both head pairs
interleaved per kt) are the ACT-bound backbone; projection and output-
projection matmul units are emitted between kt-iterations as PE filler so
the tensor engine stays busy while ACT runs exp. All PSUM flows through two
pools of [128, 1024] f32 tiles (2 banks each, bufs=2): "big" for scores /
QK-proj / V-proj / out-proj tiles, "po" for the two per-pair output
accumulators (V rows + ones row -> denominators ride along in row 64).
Warmup matmuls un-throttle the PE HAM clock gate before real work arrives;
input DMAs are column-split and priority-ordered so the first projection
starts ~4us in. Softmax normalize uses gpsimd partition_broadcast (no DRAM
round trip). Output partials stored bf16.
"""
import sys, math

sys.path.insert(0, "/opt/trn_rl_repo")

import numpy as np
import ml_dtypes

import concourse.bacc as bacc
import concourse.bass as bass
import concourse.mybir as mybir
import concourse.tile as tile
from concourse.bass_utils import run_bass_kernel_spmd

BF16 = mybir.dt.bfloat16
F32 = mybir.dt.float32
NPBF16 = ml_dtypes.bfloat16

D_MODEL = 1024
D_HEAD = 64
HALF = D_HEAD // 2
ROPE_THETA = 10000.0
N_CORES = 8
C = 256  # channels per core (4 heads x 64)
SWAP32 = [i ^ 1 for i in range(32)]


def _body(nc, tc, L, pp, rtp, ptp, rip, osp, bigp, pop):
    n_lt = L // 128
    qw = 512
    n_qch = L // qw
    scale = 1.0 / math.sqrt(D_HEAD)

    xt_d = nc.dram_tensor("xt", [D_MODEL, L], BF16, kind="ExternalInput").ap()
    wq_d = nc.dram_tensor("wqt", [D_MODEL, C], BF16, kind="ExternalInput").ap()
    wk_d = nc.dram_tensor("wkt", [D_MODEL, C], BF16, kind="ExternalInput").ap()
    wv_d = nc.dram_tensor("wvt", [D_MODEL, C], BF16, kind="ExternalInput").ap()
    wo_d = nc.dram_tensor("wot", [C, D_MODEL], BF16, kind="ExternalInput").ap()
    cos_d = nc.dram_tensor("cosb", [128, L], BF16, kind="ExternalInput").ap()
    sin_d = nc.dram_tensor("ssin", [128, L], BF16, kind="ExternalInput").ap()
    id_d = nc.dram_tensor("ident", [128, 128], BF16, kind="ExternalInput").ap()
    ng_d = nc.dram_tensor("negt", [128, 128], BF16, kind="ExternalInput").ap()
    out_d = nc.dram_tensor("out", [L, D_MODEL], BF16, kind="ExternalOutput").ap()

    # ---- persistent SBUF tensors
    wq = pp.tile([128, 8, C], BF16)
    wk = pp.tile([128, 8, C], BF16)
    wv = pp.tile([128, 8, C], BF16)
    wo = pp.tile([128, 2, D_MODEL], BF16)
    cs = pp.tile([128, L], BF16)
    sn = pp.tile([128, L], BF16)
    ident = pp.tile([128, 128], BF16)
    negt = pp.tile([128, 128], BF16)
    warm = pp.tile([128, 512], BF16)
    ones64 = pp.tile([1, 64], BF16)
    qt_c = [pp.tile([128, 2, qw], BF16, name=f"qt{i}") for i in range(n_qch)]
    kt_c = [pp.tile([128, 2, qw], BF16, name=f"ktc{i}") for i in range(n_qch)]
    # per l-tile, per head: [64 V-channels | 64 ones]; the ones columns make
    # PV deposit the softmax denominator on PSUM rows 64..127 (partition-
    # aligned with the 64 A' rows) so normalize needs no broadcast.
    vt_c = [pp.tile([128, 4, 512], BF16, name=f"vt{i}") for i in range(n_qch)]
    at = pp.tile([128, 2, L], BF16)
    xts = [pp.tile([128, L], BF16, name=f"xt{i}") for i in range(8)]

    # ---- warmup + memsets (no DMA dependency; un-throttle the PE HAM gate)
    nc.vector.memset(warm[:], 0.0)
    nc.gpsimd.memset(ones64[:], 1.0)
    for i in range(n_qch):
        ov = vt_c[i][:, :, :].rearrange("p l (h x) -> p l h x", x=128)
        nc.gpsimd.memset(ov[:, :, :, 64:128], 1.0)
    wps = bigp.tile([128, 1024], F32, tag="big", name="warm_ps")
    for r in range(16):
        nc.tensor.matmul(wps[:, 0:512], lhsT=warm[:, 0:128], rhs=warm[:],
                         start=(r == 0), stop=(r == 15), skip_group_check=True)

    # ---- input DMAs. SP queue carries weights + x in need order; the ACT
    # HWDGE queue carries the small RoPE/mask tables so they arrive early
    # without serializing behind the big transfers.
    nc.sync.dma_start(out=wq[:], in_=wq_d.rearrange("(a p) c -> p a c", p=128))
    for i in range(8):
        nc.sync.dma_start(out=xts[i][:, 0:512], in_=xt_d[i * 128:(i + 1) * 128, 0:512])
    nc.sync.dma_start(out=wk[:], in_=wk_d.rearrange("(a p) c -> p a c", p=128))
    nc.sync.dma_start(out=wv[:], in_=wv_d.rearrange("(a p) c -> p a c", p=128))
    for qc in range(1, n_qch):
        ls = qc * qw
        for i in range(8):
            nc.sync.dma_start(out=xts[i][:, ls:ls + qw],
                              in_=xt_d[i * 128:(i + 1) * 128, ls:ls + qw])
    nc.sync.dma_start(out=wo[:], in_=wo_d.rearrange("(a p) e -> p a e", p=128))
    nc.scalar.dma_start(out=cs[:, 0:512], in_=cos_d[:, 0:512])
    nc.scalar.dma_start(out=sn[:, 0:512], in_=sin_d[:, 0:512])
    nc.scalar.dma_start(out=ident[:], in_=id_d)
    nc.scalar.dma_start(out=negt[:], in_=ng_d)
    nc.scalar.dma_start(out=cs[:, 512:L], in_=cos_d[:, 512:L])
    nc.scalar.dma_start(out=sn[:, 512:L], in_=sin_d[:, 512:L])

    # ---- emission units -----------------------------------------------------
    def qk_unit(nm, w, dstc, qc):
        """Project Q^T or K^T for chunk qc (16 matmuls) + RoPE into dstc[qc]."""
        ls = qc * qw
        p = bigp.tile([128, 2, qw], F32, tag="big", name=f"ps_{nm}_{qc}")
        for ct in (0, 1):
            for dt_ in range(8):
                nc.tensor.matmul(
                    p[:, ct, :],
                    lhsT=w[:, dt_, ct * 128:ct * 128 + 128],
                    rhs=xts[dt_][:, ls:ls + qw],
                    start=(dt_ == 0), stop=(dt_ == 7), skip_group_check=True)
        dst = dstc[qc]
        for ct in (0, 1):
            sh = rtp.tile([128, qw], F32, tag="t", name=f"sh_{nm}{ct}{qc}")
            t1 = rtp.tile([128, qw], F32, tag="t", name=f"t1_{nm}{ct}{qc}")
            t2 = rtp.tile([128, qw], F32, tag="t", name=f"t2_{nm}{ct}{qc}")
            nc.vector.stream_shuffle(sh[:], p[:, ct, :], SWAP32)
            nc.vector.tensor_mul(t1[:], p[:, ct, :], cs[:, ls:ls + qw])
            nc.gpsimd.tensor_mul(t2[:], sh[:], sn[:, ls:ls + qw])
            nc.gpsimd.tensor_add(dst[:, ct, :], t1[:], t2[:])

    def v_unit(qc):
        """Project V for chunk qc's 4 l-tiles (16 matmuls) + copy to vt_c."""
        pv = bigp.tile([128, 4, C], F32, tag="big", name=f"pv_{qc}")
        for lt in range(4):
            lg = qc * 4 + lt
            for dt_ in range(8):
                nc.tensor.matmul(
                    pv[:, lt, :],
                    lhsT=xts[dt_][:, lg * 128:lg * 128 + 128],
                    rhs=wv[:, dt_, :],
                    start=(dt_ == 0), stop=(dt_ == 7), skip_group_check=True)
        ov = vt_c[qc][:, :, :].rearrange("p l (h x) -> p l h x", x=128)[:, :, :, 0:64]
        nc.vector.tensor_copy(ov, pv[:].rearrange("p l (h x) -> p l h x", x=64))

    def outproj_unit(qc, lt):
        """Output projection for l-tile qc*4+lt (4 matmuls) + store bf16."""
        lg = qc * 4 + lt
        pout = bigp.tile([128, 1024], F32, tag="big", name=f"po_{lg}")
        for ct in (0, 1):
            for eh in (0, 1):
                nc.tensor.matmul(
                    pout[:, eh * 512:eh * 512 + 512],
                    lhsT=at[:, ct, lg * 128:lg * 128 + 128],
                    rhs=wo[:, ct, eh * 512:eh * 512 + 512],
                    start=(ct == 0), stop=(ct == 1), skip_group_check=True)
        stg = osp.tile([128, 1024], BF16, tag="stg", name=f"stg_{lg}")
        nc.vector.tensor_copy(stg[:, 0:640], pout[:, 0:640])
        nc.scalar.copy(stg[:, 640:1024], pout[:, 640:1024])
        nc.sync.dma_start(out=out_d[lg * 128:lg * 128 + 128, :], in_=stg[:])

    def normalize(qc, pair, po):
        """Softmax-normalize po -> at[:, pair, chunk qc] (bf16).

        Two f32 copies drain po (PSUM freed for the next chunk's PV). The
        denominator row is broadcast to partitions 0..63 by the PE itself
        (K=1 ones matmul) — no gpsimd, so no ucode-library reloads — then
        aligned reciprocal + multiply on DVE."""
        qs = qc * qw
        rrow = rip.tile([1, 1024], BF16, tag="rrow", name=f"rr_{pair}_{qc}")
        nc.vector.tensor_copy(rrow[:], po[64:65, :])
        posb = rip.tile([64, 1024], F32, tag="posb", name=f"posb_{pair}_{qc}")
        nc.vector.tensor_copy(posb[:], po[0:64, :])
        bc = bigp.tile([128, 1024], F32, tag="big", name=f"bc_{pair}_{qc}")
        for eh in (0, 1):
            nc.tensor.matmul(bc[0:64, eh * 512:eh * 512 + 512],
                             lhsT=ones64[:], rhs=rrow[:, eh * 512:eh * 512 + 512],
                             start=True, stop=True, skip_group_check=True)
        pbi = rip.tile([64, 1024], F32, tag="pbi", name=f"pbi_{pair}_{qc}")
        nc.vector.reciprocal_approx_fast(out=pbi[:], in_=bc[0:64, :])
        tm = rip.tile([64, 1024], BF16, tag="tm", name=f"tm_{pair}_{qc}")
        nc.vector.tensor_mul(tm[:], posb[:], pbi[:])
        nc.vector.tensor_copy(at[0:64, pair, qs:qs + qw], tm[:, 0:qw])
        nc.sync.dma_start(out=at[64:128, pair, qs:qs + qw],
                          in_=tm[:, 512:512 + qw])

    # ---- fused pipeline -----------------------------------------------------
    filler = []
    qk_unit("q", wq, qt_c, 0)
    qk_unit("k", wk, kt_c, 0)
    v_unit(0)
    for qc in range(1, n_qch):
        filler.append((lambda qc=qc: qk_unit("q", wq, qt_c, qc)))
        filler.append((lambda qc=qc: qk_unit("k", wk, kt_c, qc)))
        filler.append((lambda qc=qc: v_unit(qc)))

    # filler units that MUST be emitted before att(qc) starts: proj(qc) is
    # the first 3*qc entries of the original list; track consumed count.
    consumed = [0]

    def emit_filler(n):
        for _ in range(n):
            if filler:
                filler.pop(0)()
                consumed[0] += 1

    pending = [None]

    def pv_step(po_, vt_, lt_, sb_, q0, p_, st, sp):
        for hloc in range(2):
            h = 2 * p_ + hloc
            nc.tensor.matmul(
                po_[:, 512 * hloc + q0:512 * hloc + qw],
                lhsT=vt_[:, lt_, 128 * h:128 * h + 128],
                rhs=sb_[:, 512 * hloc + q0:512 * hloc + qw],
                start=st, stop=sp,
                skip_group_check=True)

    def flush_pv():
        if pending[0] is not None:
            pending[0]()
            pending[0] = None

    for qc in range(n_qch):
        qs = qc * qw
        ktmax = (qs + qw) // 128
        po = {}
        for pair in range(2):
            po[pair] = pop.tile([128, 1024], F32, tag="po",
                                name=f"acc_{pair}_{qc}")
        for kt in range(ktmax):
            off = kt * 128 - qs
            qlo = max(0, off)
            kc, ko = kt // 4, (kt % 4) * 128
            for pair in range(2):
                pt_ps = bigp.tile([128, 1024], F32, tag="big",
                                  name=f"pt_{pair}_{qc}_{kt}")
                if not filler:
                    # HAM keeper: harmless matmuls; scores' start=True
                    # re-clears [qlo:qw], and the junk outside is never read
                    nc.tensor.matmul(
                        pt_ps[:, 0:512], lhsT=warm[:, 0:128], rhs=warm[:],
                        start=True, stop=True, skip_group_check=True)
                for hloc in range(2):
                    nc.tensor.matmul(
                        pt_ps[:, 512 * hloc + qlo:512 * hloc + qw],
                        lhsT=kt_c[kc][64 * hloc:64 * hloc + 64, pair,
                                      ko:ko + 128],
                        rhs=qt_c[qc][64 * hloc:64 * hloc + 64, pair,
                                     qlo:qw],
                        start=True, stop=(off < 0),
                        tile_position=(64 * hloc, 0),
                        skip_group_check=True)
                    if off >= 0:
                        # causal mask: accumulate -240*[q<k] onto the
                        # 128-wide diagonal block; exp then yields ~0 there
                        nc.tensor.matmul(
                            pt_ps[:, 512 * hloc + qlo:512 * hloc + qlo + 128],
                            lhsT=ident[:],
                            rhs=negt[:],
                            start=False, stop=True,
                            skip_group_check=True)
                pt_sb = ptp.tile([128, 1024], BF16, tag="p",
                                 name=f"ptsb_{pair}_{qc}_{kt}")
                pv_ps = pt_ps[:, :].rearrange("p (h x) -> p h x", h=2)
                pv_sb = pt_sb[:, :].rearrange("p (h x) -> p h x", h=2)
                nc.scalar.activation(pv_sb[:, :, qlo:qw], pv_ps[:, :, qlo:qw],
                                     mybir.ActivationFunctionType.Exp,
                                     scale=scale)
                # PV lags one (kt, pair) step: by now its exp has finished
                # during this step's scores burst, so the PE never drains
                if pending[0] is not None:
                    pending[0]()
                pending[0] = (lambda po_=po[pair], vt_=vt_c[kc], lt_=kt % 4,
                              sb_=pt_sb, q0=qlo, p_=pair,
                              st=(kt == 0), sp=(kt == ktmax - 1):
                              pv_step(po_, vt_, lt_, sb_, q0, p_, st, sp))
                emit_filler(1)
        flush_pv()
        for pair in range(2):
            normalize(qc, pair, po[pair])
        if qc + 1 < n_qch:
            drain = 3 * (qc + 1) - consumed[0]
            emit_filler(max(0, drain))
        for lt in range(4):
            filler.append((lambda qc=qc, lt=lt: outproj_unit(qc, lt)))
    emit_filler(len(filler))


def build_nc(L=2048):
    """Build + compile the per-core Bass program (same NEFF on all 8 cores)."""
    assert L % 512 == 0
    nc = bacc.Bacc("TRN2", target_bir_lowering=False, debug=False,
                   num_devices=N_CORES)
    with tile.TileContext(nc) as tc:
        with tc.tile_pool(name="persist", bufs=1) as pp, \
             tc.tile_pool(name="ropet", bufs=6) as rtp, \
             tc.tile_pool(name="ptp", bufs=3) as ptp, \
             tc.tile_pool(name="rinvp", bufs=2) as rip, \
             tc.tile_pool(name="ostg", bufs=3) as osp, \
             tc.tile_pool(name="bigps", bufs=2, space="PSUM") as bigp, \
             tc.tile_pool(name="pops", bufs=2, space="PSUM") as pop:
            _body(nc, tc, L, pp, rtp, ptp, rip, osp, bigp, pop)
    nc.compile()
    return nc


_NC_CACHE = {}


def _get_nc(L):
    if L not in _NC_CACHE:
        _NC_CACHE[L] = build_nc(L)
    return _NC_CACHE[L]


def make_inputs(x, token_positions, Wq, Wk, Wv, Wo):
    """Host-side shard/layout prep -> list of 8 per-core input dicts."""
    B, L, _ = x.shape
    pos = np.asarray(token_positions).astype(np.float64)
    S = ROPE_THETA ** (-2.0 / D_HEAD)
    thetas = S ** np.arange(HALF, dtype=np.float64)
    ang = pos[:, None] * thetas[None, :]          # [L, 32]
    cosL = np.cos(ang).T                          # [32, L]
    sinL = np.sin(ang).T
    # per-channel tables on the natural (head, dim) layout:
    # row p (within a 64-row head block): pair i = (p%64)//2
    # cosb[p] = cos(theta_i * pos); ssin[p] = -sin if dim even else +sin
    cosb = np.empty((128, L), dtype=np.float64)
    ssin = np.empty((128, L), dtype=np.float64)
    for p in range(128):
        i = (p % 64) // 2
        cosb[p] = cosL[i]
        ssin[p] = -sinL[i] if (p % 2 == 0) else sinL[i]
    cosb = cosb.astype(NPBF16)
    ssin = ssin.astype(NPBF16)

    r = np.arange(128)[:, None]
    col = np.arange(128)[None, :]
    ident = np.eye(128, dtype=NPBF16)
    negt = np.where(col < r, -240.0, 0.0).astype(NPBF16)  # pre-exp mask bias

    xts = [np.ascontiguousarray(x[b].astype(NPBF16).T) for b in range(B)]
    in_maps = []
    shard_cache = {}
    for core in range(N_CORES):
        b, hg = core // 4, core % 4
        if hg not in shard_cache:
            rows = slice(hg * 256, hg * 256 + 256)
            shard_cache[hg] = {
                "wqt": np.ascontiguousarray(Wq[rows].astype(NPBF16).T),
                "wkt": np.ascontiguousarray(Wk[rows].astype(NPBF16).T),
                "wvt": np.ascontiguousarray(Wv[rows].astype(NPBF16).T),
                "wot": np.ascontiguousarray(Wo[:, rows].astype(NPBF16).T),
            }
        m = dict(shard_cache[hg])
        m["xt"] = xts[b]
        m["cosb"] = cosb
        m["ssin"] = ssin
        m["ident"] = ident
        m["negt"] = negt
        in_maps.append(m)
    return in_maps


def kernel(x, token_positions, Wq, Wk, Wv, Wo):
    x = np.asarray(x); Wq = np.asarray(Wq); Wk = np.asarray(Wk)
    Wv = np.asarray(Wv); Wo = np.asarray(Wo)
    B, L, _ = x.shape
    nc = _get_nc(L)
    in_maps = make_inputs(x, token_positions, Wq, Wk, Wv, Wo)
    res = run_bass_kernel_spmd(nc, in_maps, core_ids=list(range(N_CORES)))
    out = np.zeros((B, L, D_MODEL), dtype=np.float32)
    for core in range(N_CORES):
        out[core // 4] += res.results[core]["out"].astype(np.float32)
    return out


# revision 40
# speedup vs baseline: 1.2123x; 1.0052x over previous
"""Multi-head self-attention (RoPE, causal) Trainium2 Bass kernel, 8 NeuronCores.

Sharding: data-parallel over batch (B=2) x tensor-parallel over heads
(16 heads -> 4 groups of 4). Core c handles batch b=c//4, heads 4*(c%4)..4*(c%4)+3.
Each core computes its 4 heads' attention plus a partial output projection;
the host sums the 4 partial outputs per batch element (bf16 partials).

Single fused pipeline (v2): attention kt-iterations (both head pairs
interleaved per kt) are the ACT-bound backbone; projection and output-
projection matmul units are emitted between kt-iterations as PE filler so
the tensor engine stays busy while ACT runs exp. All PSUM flows through two
pools of [128, 1024] f32 tiles (2 banks each, bufs=2): "big" for scores /
QK-proj / V-proj / out-proj tiles, "po" for the two per-pair output
accumulators (V rows + ones row -> denominators ride along in row 64).
Warmup matmuls un-throttle the PE HAM clock gate before real work arrives;
input DMAs are column-split and priority-ordered so the first projection
starts ~4us in. Softmax normalize uses gpsimd partition_broadcast (no DRAM
round trip). Output partials stored bf16.
"""
import sys, math

sys.path.insert(0, "/opt/trn_rl_repo")

import numpy as np
import ml_dtypes

import concourse.bacc as bacc
import concourse.bass as bass
import concourse.mybir as mybir
import concourse.tile as tile
from concourse.bass_utils import run_bass_kernel_spmd

BF16 = mybir.dt.bfloat16
F32 = mybir.dt.float32
NPBF16 = ml_dtypes.bfloat16

D_MODEL = 1024
D_HEAD = 64
HALF = D_HEAD // 2
ROPE_THETA = 10000.0
N_CORES = 8
C = 256  # channels per core (4 heads x 64)
SWAP32 = [i ^ 1 for i in range(32)]


def _body(nc, tc, L, pp, rtp, ptp, rip, osp, bigp, pop, scrp):
    n_lt = L // 128
    qw = 512
    n_qch = L // qw
    scale = 1.0 / math.sqrt(D_HEAD)

    xt_d = nc.dram_tensor("xt", [D_MODEL, L], BF16, kind="ExternalInput").ap()
    wq_d = nc.dram_tensor("wqt", [D_MODEL, C], BF16, kind="ExternalInput").ap()
    wk_d = nc.dram_tensor("wkt", [D_MODEL, C], BF16, kind="ExternalInput").ap()
    wv_d = nc.dram_tensor("wvt", [D_MODEL, C], BF16, kind="ExternalInput").ap()
    wo_d = nc.dram_tensor("wot", [C, D_MODEL], BF16, kind="ExternalInput").ap()
    cos_d = nc.dram_tensor("cosb", [128, L], BF16, kind="ExternalInput").ap()
    sin_d = nc.dram_tensor("ssin", [128, L], BF16, kind="ExternalInput").ap()
    id_d = nc.dram_tensor("ident", [128, 128], BF16, kind="ExternalInput").ap()
    ng_d = nc.dram_tensor("negt", [128, 128], BF16, kind="ExternalInput").ap()
    out_d = nc.dram_tensor("out", [L, D_MODEL], BF16, kind="ExternalOutput").ap()

    # ---- persistent SBUF tensors
    wq = pp.tile([128, 8, C], BF16)
    wk = pp.tile([128, 8, C], BF16)
    wv = pp.tile([128, 8, C], BF16)
    wo = pp.tile([128, 2, D_MODEL], BF16)
    cs = pp.tile([128, L], BF16)
    sn = pp.tile([128, L], BF16)
    ident = pp.tile([128, 128], BF16)
    negt = pp.tile([128, 128], BF16)
    warm = pp.tile([128, 512], BF16)
    qt_c = [pp.tile([128, 2, qw], BF16, name=f"qt{i}") for i in range(n_qch)]
    kt_c = [pp.tile([128, 2, qw], BF16, name=f"ktc{i}") for i in range(n_qch)]
    # per l-tile, per head: [64 V-channels | 64 ones]; the ones columns make
    # PV deposit the softmax denominator on PSUM rows 64..127 (partition-
    # aligned with the 64 A' rows) so normalize needs no broadcast.
    vt_c = [pp.tile([128, 4, 512], BF16, name=f"vt{i}") for i in range(n_qch)]
    at = pp.tile([128, 2, L], BF16)
    xts = [pp.tile([128, L], BF16, name=f"xt{i}") for i in range(8)]

    # ---- warmup + memsets (no DMA dependency; un-throttle the PE HAM gate)
    nc.vector.memset(warm[:], 0.0)
    for i in range(n_qch):
        ov = vt_c[i][:, :, :].rearrange("p l (h x) -> p l h x", x=128)
        nc.gpsimd.memset(ov[:, :, :, 64:128], 1.0)
    wps = bigp.tile([128, 1024], F32, tag="big", name="warm_ps")
    for r in range(16):
        nc.tensor.matmul(wps[:, 0:512], lhsT=warm[:, 0:128], rhs=warm[:],
                         start=(r == 0), stop=(r == 15), skip_group_check=True)

    # ---- input DMAs. SP queue carries weights + x in need order; the ACT
    # HWDGE queue carries the small RoPE/mask tables so they arrive early
    # without serializing behind the big transfers.
    nc.sync.dma_start(out=wq[:], in_=wq_d.rearrange("(a p) c -> p a c", p=128))
    for i in range(8):
        nc.sync.dma_start(out=xts[i][:, 0:512], in_=xt_d[i * 128:(i + 1) * 128, 0:512])
    nc.sync.dma_start(out=wk[:], in_=wk_d.rearrange("(a p) c -> p a c", p=128))
    nc.sync.dma_start(out=wv[:], in_=wv_d.rearrange("(a p) c -> p a c", p=128))
    for qc in range(1, n_qch):
        ls = qc * qw
        for i in range(8):
            nc.sync.dma_start(out=xts[i][:, ls:ls + qw],
                              in_=xt_d[i * 128:(i + 1) * 128, ls:ls + qw])
    nc.sync.dma_start(out=wo[:], in_=wo_d.rearrange("(a p) e -> p a e", p=128))
    nc.scalar.dma_start(out=cs[:, 0:512], in_=cos_d[:, 0:512])
    nc.scalar.dma_start(out=sn[:, 0:512], in_=sin_d[:, 0:512])
    nc.scalar.dma_start(out=ident[:], in_=id_d)
    nc.scalar.dma_start(out=negt[:], in_=ng_d)
    nc.scalar.dma_start(out=cs[:, 512:L], in_=cos_d[:, 512:L])
    nc.scalar.dma_start(out=sn[:, 512:L], in_=sin_d[:, 512:L])

    # ---- emission units -----------------------------------------------------
    def qk_unit(nm, w, dstc, qc):
        """Project Q^T or K^T for chunk qc (16 matmuls) + RoPE into dstc[qc]."""
        ls = qc * qw
        p = bigp.tile([128, 2, qw], F32, tag="big", name=f"ps_{nm}_{qc}")
        for ct in (0, 1):
            for dt_ in range(8):
                nc.tensor.matmul(
                    p[:, ct, :],
                    lhsT=w[:, dt_, ct * 128:ct * 128 + 128],
                    rhs=xts[dt_][:, ls:ls + qw],
                    start=(dt_ == 0), stop=(dt_ == 7), skip_group_check=True)
        dst = dstc[qc]
        for ct in (0, 1):
            sh = rtp.tile([128, qw], F32, tag="t", name=f"sh_{nm}{ct}{qc}")
            t1 = rtp.tile([128, qw], F32, tag="t", name=f"t1_{nm}{ct}{qc}")
            t2 = rtp.tile([128, qw], F32, tag="t", name=f"t2_{nm}{ct}{qc}")
            nc.vector.stream_shuffle(sh[:], p[:, ct, :], SWAP32)
            nc.vector.tensor_mul(t1[:], p[:, ct, :], cs[:, ls:ls + qw])
            nc.gpsimd.tensor_mul(t2[:], sh[:], sn[:, ls:ls + qw])
            nc.gpsimd.tensor_add(dst[:, ct, :], t1[:], t2[:])

    def v_unit(qc):
        """Project V for chunk qc's 4 l-tiles (16 matmuls) + copy to vt_c."""
        pv = bigp.tile([128, 4, C], F32, tag="big", name=f"pv_{qc}")
        for lt in range(4):
            lg = qc * 4 + lt
            for dt_ in range(8):
                nc.tensor.matmul(
                    pv[:, lt, :],
                    lhsT=xts[dt_][:, lg * 128:lg * 128 + 128],
                    rhs=wv[:, dt_, :],
                    start=(dt_ == 0), stop=(dt_ == 7), skip_group_check=True)
        ov = vt_c[qc][:, :, :].rearrange("p l (h x) -> p l h x", x=128)[:, :, :, 0:64]
        nc.vector.tensor_copy(ov, pv[:].rearrange("p l (h x) -> p l h x", x=64))

    def outproj_unit(qc, lt):
        """Output projection for l-tile qc*4+lt (4 matmuls) + store bf16."""
        lg = qc * 4 + lt
        pout = bigp.tile([128, 1024], F32, tag="big", name=f"po_{lg}")
        for ct in (0, 1):
            for eh in (0, 1):
                nc.tensor.matmul(
                    pout[:, eh * 512:eh * 512 + 512],
                    lhsT=at[:, ct, lg * 128:lg * 128 + 128],
                    rhs=wo[:, ct, eh * 512:eh * 512 + 512],
                    start=(ct == 0), stop=(ct == 1), skip_group_check=True)
        stg = osp.tile([128, 1024], BF16, tag="stg", name=f"stg_{lg}")
        nc.vector.tensor_copy(stg[:, 0:640], pout[:, 0:640])
        nc.scalar.copy(stg[:, 640:1024], pout[:, 640:1024])
        nc.sync.dma_start(out=out_d[lg * 128:lg * 128 + 128, :], in_=stg[:])

    def normalize(qc, pair, po):
        """Softmax-normalize po -> at[:, pair, chunk qc] (bf16).

        Two f32 copies drain po (PSUM freed for the next chunk's PV in ~1us).
        The denominator row is broadcast to partitions 0..63 via a DRAM
        DMA bounce — off every compute queue, so the next chunk's PE stream
        never waits on it — then aligned reciprocal + multiply on DVE."""
        qs = qc * qw
        rrow = rip.tile([1, 1024], F32, tag="rrow", name=f"rr_{pair}_{qc}")
        nc.vector.tensor_copy(rrow[:], po[64:65, :])
        posb = rip.tile([64, 1024], F32, tag="posb", name=f"posb_{pair}_{qc}")
        nc.vector.tensor_copy(posb[:], po[0:64, :])
        scr = scrp.tile([1, 1024], F32, tag="scr", name=f"scr_{pair}_{qc}")
        nc.sync.dma_start(out=scr[:], in_=rrow[:])
        pb0 = rip.tile([64, 1024], F32, tag="pb0", name=f"pb0_{pair}_{qc}")
        nc.sync.dma_start(out=pb0[:], in_=scr[:].partition_broadcast(64))
        pbi = rip.tile([64, 1024], F32, tag="pbi", name=f"pbi_{pair}_{qc}")
        nc.vector.reciprocal_approx_fast(out=pbi[:], in_=pb0[:])
        tm = rip.tile([64, 1024], BF16, tag="tm", name=f"tm_{pair}_{qc}")
        nc.vector.tensor_mul(tm[:], posb[:], pbi[:])
        nc.vector.tensor_copy(at[0:64, pair, qs:qs + qw], tm[:, 0:qw])
        nc.sync.dma_start(out=at[64:128, pair, qs:qs + qw],
                          in_=tm[:, 512:512 + qw])

    # ---- fused pipeline -----------------------------------------------------
    filler = []
    qk_unit("q", wq, qt_c, 0)
    qk_unit("k", wk, kt_c, 0)
    v_unit(0)
    # bridge the proj(0) RoPE latency so the HAM window never sees idle
    brg = bigp.tile([128, 1024], F32, tag="big", name="bridge_ps")
    for r in range(8):
        nc.tensor.matmul(brg[:, 0:512], lhsT=warm[:, 0:128], rhs=warm[:],
                         start=(r == 0), stop=(r == 7), skip_group_check=True)
    for qc in range(1, n_qch):
        filler.append((lambda qc=qc: qk_unit("q", wq, qt_c, qc)))
        filler.append((lambda qc=qc: qk_unit("k", wk, kt_c, qc)))
        filler.append((lambda qc=qc: v_unit(qc)))

    # filler units that MUST be emitted before att(qc) starts: proj(qc) is
    # the first 3*qc entries of the original list; track consumed count.
    consumed = [0]

    def emit_filler(n):
        for _ in range(n):
            if filler:
                filler.pop(0)()
                consumed[0] += 1

    pending = [None]

    def pv_step(po_, vt_, lt_, sb_, q0, p_, st, sp):
        for hloc in range(2):
            h = 2 * p_ + hloc
            nc.tensor.matmul(
                po_[:, 512 * hloc + q0:512 * hloc + qw],
                lhsT=vt_[:, lt_, 128 * h:128 * h + 128],
                rhs=sb_[:, 512 * hloc + q0:512 * hloc + qw],
                start=st, stop=sp,
                skip_group_check=True)

    def flush_pv():
        if pending[0] is not None:
            pending[0]()
            pending[0] = None

    for qc in range(n_qch):
        qs = qc * qw
        ktmax = (qs + qw) // 128
        po = {}
        for pair in range(2):
            po[pair] = pop.tile([128, 1024], F32, tag="po",
                                name=f"acc_{pair}_{qc}")
        for kt in range(ktmax):
            off = kt * 128 - qs
            qlo = max(0, off)
            kc, ko = kt // 4, (kt % 4) * 128
            for pair in range(2):
                pt_ps = bigp.tile([128, 1024], F32, tag="big",
                                  name=f"pt_{pair}_{qc}_{kt}")
                if not filler:
                    # HAM keeper: harmless matmuls; scores' start=True
                    # re-clears [qlo:qw], and the junk outside is never read
                    nc.tensor.matmul(
                        pt_ps[:, 0:512], lhsT=warm[:, 0:128], rhs=warm[:],
                        start=True, stop=True, skip_group_check=True)
                for hloc in range(2):
                    nc.tensor.matmul(
                        pt_ps[:, 512 * hloc + qlo:512 * hloc + qw],
                        lhsT=kt_c[kc][64 * hloc:64 * hloc + 64, pair,
                                      ko:ko + 128],
                        rhs=qt_c[qc][64 * hloc:64 * hloc + 64, pair,
                                     qlo:qw],
                        start=True, stop=(off < 0),
                        tile_position=(64 * hloc, 0),
                        skip_group_check=True)
                    if off >= 0:
                        # causal mask: accumulate -240*[q<k] onto the
                        # 128-wide diagonal block; exp then yields ~0 there
                        nc.tensor.matmul(
                            pt_ps[:, 512 * hloc + qlo:512 * hloc + qlo + 128],
                            lhsT=ident[:],
                            rhs=negt[:],
                            start=False, stop=True,
                            skip_group_check=True)
                pt_sb = ptp.tile([128, 1024], BF16, tag="p",
                                 name=f"ptsb_{pair}_{qc}_{kt}")
                pv_ps = pt_ps[:, :].rearrange("p (h x) -> p h x", h=2)
                pv_sb = pt_sb[:, :].rearrange("p (h x) -> p h x", h=2)
                nc.scalar.activation(pv_sb[:, :, qlo:qw], pv_ps[:, :, qlo:qw],
                                     mybir.ActivationFunctionType.Exp,
                                     scale=scale)
                # PV lags one (kt, pair) step: by now its exp has finished
                # during this step's scores burst, so the PE never drains
                if pending[0] is not None:
                    pending[0]()
                pending[0] = (lambda po_=po[pair], vt_=vt_c[kc], lt_=kt % 4,
                              sb_=pt_sb, q0=qlo, p_=pair,
                              st=(kt == 0), sp=(kt == ktmax - 1):
                              pv_step(po_, vt_, lt_, sb_, q0, p_, st, sp))
                emit_filler(1)
        flush_pv()
        for pair in range(2):
            normalize(qc, pair, po[pair])
        if qc + 1 < n_qch:
            drain = 3 * (qc + 1) - consumed[0]
            emit_filler(max(0, drain))
        for lt in range(4):
            filler.append((lambda qc=qc, lt=lt: outproj_unit(qc, lt)))
    emit_filler(len(filler))


def build_nc(L=2048):
    """Build + compile the per-core Bass program (same NEFF on all 8 cores)."""
    assert L % 512 == 0
    nc = bacc.Bacc("TRN2", target_bir_lowering=False, debug=False,
                   num_devices=N_CORES)
    with tile.TileContext(nc) as tc:
        with tc.tile_pool(name="persist", bufs=1) as pp, \
             tc.tile_pool(name="ropet", bufs=6) as rtp, \
             tc.tile_pool(name="ptp", bufs=3) as ptp, \
             tc.tile_pool(name="rinvp", bufs=2) as rip, \
             tc.tile_pool(name="ostg", bufs=3) as osp, \
             tc.tile_pool(name="bigps", bufs=2, space="PSUM") as bigp, \
             tc.tile_pool(name="pops", bufs=2, space="PSUM") as pop, \
             tc.tile_pool(name="riscr", bufs=4, space="DRAM") as scrp:
            _body(nc, tc, L, pp, rtp, ptp, rip, osp, bigp, pop, scrp)
    nc.compile()
    return nc


_NC_CACHE = {}


def _get_nc(L):
    if L not in _NC_CACHE:
        _NC_CACHE[L] = build_nc(L)
    return _NC_CACHE[L]


def make_inputs(x, token_positions, Wq, Wk, Wv, Wo):
    """Host-side shard/layout prep -> list of 8 per-core input dicts."""
    B, L, _ = x.shape
    pos = np.asarray(token_positions).astype(np.float64)
    S = ROPE_THETA ** (-2.0 / D_HEAD)
    thetas = S ** np.arange(HALF, dtype=np.float64)
    ang = pos[:, None] * thetas[None, :]          # [L, 32]
    cosL = np.cos(ang).T                          # [32, L]
    sinL = np.sin(ang).T
    # per-channel tables on the natural (head, dim) layout:
    # row p (within a 64-row head block): pair i = (p%64)//2
    # cosb[p] = cos(theta_i * pos); ssin[p] = -sin if dim even else +sin
    cosb = np.empty((128, L), dtype=np.float64)
    ssin = np.empty((128, L), dtype=np.float64)
    for p in range(128):
        i = (p % 64) // 2
        cosb[p] = cosL[i]
        ssin[p] = -sinL[i] if (p % 2 == 0) else sinL[i]
    cosb = cosb.astype(NPBF16)
    ssin = ssin.astype(NPBF16)

    r = np.arange(128)[:, None]
    col = np.arange(128)[None, :]
    ident = np.eye(128, dtype=NPBF16)
    negt = np.where(col < r, -240.0, 0.0).astype(NPBF16)  # pre-exp mask bias

    xts = [np.ascontiguousarray(x[b].astype(NPBF16).T) for b in range(B)]
    in_maps = []
    shard_cache = {}
    for core in range(N_CORES):
        b, hg = core // 4, core % 4
        if hg not in shard_cache:
            rows = slice(hg * 256, hg * 256 + 256)
            shard_cache[hg] = {
                "wqt": np.ascontiguousarray(Wq[rows].astype(NPBF16).T),
                "wkt": np.ascontiguousarray(Wk[rows].astype(NPBF16).T),
                "wvt": np.ascontiguousarray(Wv[rows].astype(NPBF16).T),
                "wot": np.ascontiguousarray(Wo[:, rows].astype(NPBF16).T),
            }
        m = dict(shard_cache[hg])
        m["xt"] = xts[b]
        m["cosb"] = cosb
        m["ssin"] = ssin
        m["ident"] = ident
        m["negt"] = negt
        in_maps.append(m)
    return in_maps


def kernel(x, token_positions, Wq, Wk, Wv, Wo):
    x = np.asarray(x); Wq = np.asarray(Wq); Wk = np.asarray(Wk)
    Wv = np.asarray(Wv); Wo = np.asarray(Wo)
    B, L, _ = x.shape
    nc = _get_nc(L)
    in_maps = make_inputs(x, token_positions, Wq, Wk, Wv, Wo)
    res = run_bass_kernel_spmd(nc, in_maps, core_ids=list(range(N_CORES)))
    out = np.zeros((B, L, D_MODEL), dtype=np.float32)
    for core in range(N_CORES):
        out[core // 4] += res.results[core]["out"].astype(np.float32)
    return out


# revision 46
# speedup vs baseline: 1.2386x; 1.0217x over previous
"""Multi-head self-attention (RoPE, causal) Trainium2 Bass kernel, 8 NeuronCores.

Sharding: data-parallel over batch (B=2) x tensor-parallel over heads
(16 heads -> 4 groups of 4). Core c handles batch b=c//4, heads 4*(c%4)..4*(c%4)+3.
Each core computes its 4 heads' attention plus a partial output projection;
the host sums the 4 partial outputs per batch element (bf16 partials).

Single fused pipeline (v2): attention kt-iterations (both head pairs
interleaved per kt) are the ACT-bound backbone; projection and output-
projection matmul units are emitted between kt-iterations as PE filler so
the tensor engine stays busy while ACT runs exp. All PSUM flows through two
pools of [128, 1024] f32 tiles (2 banks each, bufs=2): "big" for scores /
QK-proj / V-proj / out-proj tiles, "po" for the two per-pair output
accumulators (V rows + ones row -> denominators ride along in row 64).
Warmup matmuls un-throttle the PE HAM clock gate before real work arrives;
input DMAs are column-split and priority-ordered so the first projection
starts ~4us in. Softmax normalize uses gpsimd partition_broadcast (no DRAM
round trip). Output partials stored bf16.
"""
import sys, math

sys.path.insert(0, "/opt/trn_rl_repo")

import numpy as np
import ml_dtypes

import concourse.bacc as bacc
import concourse.bass as bass
import concourse.mybir as mybir
import concourse.tile as tile
from concourse.bass_utils import run_bass_kernel_spmd

BF16 = mybir.dt.bfloat16
F32 = mybir.dt.float32
NPBF16 = ml_dtypes.bfloat16

D_MODEL = 1024
D_HEAD = 64
HALF = D_HEAD // 2
ROPE_THETA = 10000.0
N_CORES = 8
C = 256  # channels per core (4 heads x 64)
SWAP32 = [i ^ 1 for i in range(32)]


def _body(nc, tc, L, pp, rtp, ptp, rip, osp, bigp, pop, scrp):
    n_lt = L // 128
    qw = 512
    n_qch = L // qw
    scale = 1.0 / math.sqrt(D_HEAD)

    xt_d = nc.dram_tensor("xt", [D_MODEL, L], BF16, kind="ExternalInput").ap()
    wq_d = nc.dram_tensor("wqt", [D_MODEL, C], BF16, kind="ExternalInput").ap()
    wk_d = nc.dram_tensor("wkt", [D_MODEL, C], BF16, kind="ExternalInput").ap()
    wv_d = nc.dram_tensor("wvt", [D_MODEL, C], BF16, kind="ExternalInput").ap()
    wo_d = nc.dram_tensor("wot", [C, D_MODEL], BF16, kind="ExternalInput").ap()
    cos_d = nc.dram_tensor("cosb", [128, L], BF16, kind="ExternalInput").ap()
    sin_d = nc.dram_tensor("ssin", [128, L], BF16, kind="ExternalInput").ap()
    id_d = nc.dram_tensor("ident", [128, 128], BF16, kind="ExternalInput").ap()
    ng_d = nc.dram_tensor("negt", [128, 128], BF16, kind="ExternalInput").ap()
    out_d = nc.dram_tensor("out", [L, D_MODEL], BF16, kind="ExternalOutput").ap()

    # ---- persistent SBUF tensors
    wq = pp.tile([128, 8, C], BF16)
    wk = pp.tile([128, 8, C], BF16)
    wv = pp.tile([128, 8, C], BF16)
    wo = pp.tile([128, 2, D_MODEL], BF16)
    cs = pp.tile([128, L], BF16)
    sn = pp.tile([128, L], BF16)
    ident = pp.tile([128, 128], BF16)
    negt = pp.tile([128, 128], BF16)
    warm = pp.tile([128, 512], BF16)
    qt_c = [pp.tile([128, 2, qw], BF16, name=f"qt{i}") for i in range(n_qch)]
    kt_c = [pp.tile([128, 2, qw], BF16, name=f"ktc{i}") for i in range(n_qch)]
    # per l-tile, per head: [64 V-channels | 64 ones]; the ones columns make
    # PV deposit the softmax denominator on PSUM rows 64..127 (partition-
    # aligned with the 64 A' rows) so normalize needs no broadcast.
    vt_c = [pp.tile([128, 4, 512], BF16, name=f"vt{i}") for i in range(n_qch)]
    at = pp.tile([128, 2, L], BF16)
    xts = [pp.tile([128, L], BF16, name=f"xt{i}") for i in range(8)]

    # ---- warmup + memsets (no DMA dependency; un-throttle the PE HAM gate)
    nc.vector.memset(warm[:], 0.0)
    for i in range(n_qch):
        ov = vt_c[i][:, :, :].rearrange("p l (h x) -> p l h x", x=128)
        nc.gpsimd.memset(ov[:, :, :, 64:128], 1.0)
    wps = bigp.tile([128, 1024], F32, tag="big", name="warm_ps")
    for r in range(16):
        nc.tensor.matmul(wps[:, 0:512], lhsT=warm[:, 0:128], rhs=warm[:],
                         start=(r == 0), stop=(r == 15), skip_group_check=True)

    # ---- input DMAs. SP queue carries weights + x in need order; the ACT
    # HWDGE queue carries the small RoPE/mask tables so they arrive early
    # without serializing behind the big transfers.
    nc.sync.dma_start(out=wq[:], in_=wq_d.rearrange("(a p) c -> p a c", p=128))
    for i in range(8):
        nc.sync.dma_start(out=xts[i][:, 0:512], in_=xt_d[i * 128:(i + 1) * 128, 0:512])
    nc.sync.dma_start(out=wk[:], in_=wk_d.rearrange("(a p) c -> p a c", p=128))
    nc.sync.dma_start(out=wv[:], in_=wv_d.rearrange("(a p) c -> p a c", p=128))
    for qc in range(1, n_qch):
        ls = qc * qw
        for i in range(8):
            nc.sync.dma_start(out=xts[i][:, ls:ls + qw],
                              in_=xt_d[i * 128:(i + 1) * 128, ls:ls + qw])
    nc.sync.dma_start(out=wo[:], in_=wo_d.rearrange("(a p) e -> p a e", p=128))
    nc.scalar.dma_start(out=cs[:, 0:512], in_=cos_d[:, 0:512])
    nc.scalar.dma_start(out=sn[:, 0:512], in_=sin_d[:, 0:512])
    nc.scalar.dma_start(out=ident[:], in_=id_d)
    nc.scalar.dma_start(out=negt[:], in_=ng_d)
    nc.scalar.dma_start(out=cs[:, 512:L], in_=cos_d[:, 512:L])
    nc.scalar.dma_start(out=sn[:, 512:L], in_=sin_d[:, 512:L])

    # ---- emission units -----------------------------------------------------
    def qk_unit(nm, w, dstc, qc):
        """Project Q^T or K^T for chunk qc (16 matmuls) + RoPE into dstc[qc]."""
        ls = qc * qw
        p = bigp.tile([128, 2, qw], F32, tag="big", name=f"ps_{nm}_{qc}")
        for ct in (0, 1):
            for dt_ in range(8):
                nc.tensor.matmul(
                    p[:, ct, :],
                    lhsT=w[:, dt_, ct * 128:ct * 128 + 128],
                    rhs=xts[dt_][:, ls:ls + qw],
                    start=(dt_ == 0), stop=(dt_ == 7), skip_group_check=True)
        dst = dstc[qc]
        for ct in (0, 1):
            sh = rtp.tile([128, qw], F32, tag="t", name=f"sh_{nm}{ct}{qc}")
            t1 = rtp.tile([128, qw], F32, tag="t", name=f"t1_{nm}{ct}{qc}")
            t2 = rtp.tile([128, qw], F32, tag="t", name=f"t2_{nm}{ct}{qc}")
            nc.vector.stream_shuffle(sh[:], p[:, ct, :], SWAP32)
            nc.vector.tensor_mul(t1[:], p[:, ct, :], cs[:, ls:ls + qw])
            nc.gpsimd.tensor_mul(t2[:], sh[:], sn[:, ls:ls + qw])
            nc.gpsimd.tensor_add(dst[:, ct, :], t1[:], t2[:])

    def v_unit(qc):
        """Project V for chunk qc's 4 l-tiles (16 matmuls) + copy to vt_c."""
        pv = bigp.tile([128, 4, C], F32, tag="big", name=f"pv_{qc}")
        for lt in range(4):
            lg = qc * 4 + lt
            for dt_ in range(8):
                nc.tensor.matmul(
                    pv[:, lt, :],
                    lhsT=xts[dt_][:, lg * 128:lg * 128 + 128],
                    rhs=wv[:, dt_, :],
                    start=(dt_ == 0), stop=(dt_ == 7), skip_group_check=True)
        ov = vt_c[qc][:, :, :].rearrange("p l (h x) -> p l h x", x=128)[:, :, :, 0:64]
        nc.vector.tensor_copy(ov, pv[:].rearrange("p l (h x) -> p l h x", x=64))

    def outproj_unit(qc, lt):
        """Output projection for l-tile qc*4+lt (4 matmuls) + store bf16."""
        lg = qc * 4 + lt
        pout = bigp.tile([128, 1024], F32, tag="big", name=f"po_{lg}")
        for ct in (0, 1):
            for eh in (0, 1):
                nc.tensor.matmul(
                    pout[:, eh * 512:eh * 512 + 512],
                    lhsT=at[:, ct, lg * 128:lg * 128 + 128],
                    rhs=wo[:, ct, eh * 512:eh * 512 + 512],
                    start=(ct == 0), stop=(ct == 1), skip_group_check=True)
        stg = osp.tile([128, 1024], BF16, tag="stg", name=f"stg_{lg}")
        nc.scalar.copy(stg[:, 0:512], pout[:, 0:512])
        nc.scalar.copy(stg[:, 512:1024], pout[:, 512:1024])
        nc.sync.dma_start(out=out_d[lg * 128:lg * 128 + 128, :], in_=stg[:])

    def normalize(qc, pair, po):
        """Softmax-normalize po -> at[:, pair, chunk qc] (bf16).

        Two f32 copies drain po (PSUM freed for the next chunk's PV in ~1us).
        The denominator row is broadcast to partitions 0..63 via a DRAM
        DMA bounce — off every compute queue, so the next chunk's PE stream
        never waits on it — then aligned reciprocal + multiply on DVE."""
        qs = qc * qw
        rrow = rip.tile([1, 1024], F32, tag="rrow", name=f"rr_{pair}_{qc}")
        nc.scalar.copy(rrow[:], po[64:65, :])
        posb = rip.tile([64, 1024], F32, tag="posb", name=f"posb_{pair}_{qc}")
        nc.vector.tensor_copy(posb[:], po[0:64, :])
        scr = scrp.tile([1, 1024], F32, tag="scr", name=f"scr_{pair}_{qc}")
        nc.scalar.dma_start(out=scr[:], in_=rrow[:])
        pb0 = rip.tile([64, 1024], F32, tag="pb0", name=f"pb0_{pair}_{qc}")
        nc.scalar.dma_start(out=pb0[:], in_=scr[:].partition_broadcast(64))
        pbi = rip.tile([64, 1024], F32, tag="pbi", name=f"pbi_{pair}_{qc}")
        nc.vector.reciprocal_approx_fast(out=pbi[:], in_=pb0[:])
        tm = rip.tile([64, 1024], BF16, tag="tm", name=f"tm_{pair}_{qc}")
        nc.vector.tensor_mul(tm[:], posb[:], pbi[:])
        nc.vector.tensor_copy(at[0:64, pair, qs:qs + qw], tm[:, 0:qw])
        nc.scalar.dma_start(out=at[64:128, pair, qs:qs + qw],
                            in_=tm[:, 512:512 + qw])

    # ---- fused pipeline -----------------------------------------------------
    filler = []  # entries: (gate_step, fn)
    step = [0]
    qk_unit("q", wq, qt_c, 0)
    qk_unit("k", wk, kt_c, 0)
    v_unit(0)
    # bridge the proj(0) RoPE latency so the HAM window never sees idle
    brg = bigp.tile([128, 1024], F32, tag="big", name="bridge_ps")
    for r in range(8):
        nc.tensor.matmul(brg[:, 0:512], lhsT=warm[:, 0:128], rhs=warm[:],
                         start=(r == 0), stop=(r == 7), skip_group_check=True)
    for qc in range(1, n_qch):
        filler.append((0, lambda qc=qc: qk_unit("q", wq, qt_c, qc)))
        filler.append((0, lambda qc=qc: qk_unit("k", wk, kt_c, qc)))
        filler.append((0, lambda qc=qc: v_unit(qc)))

    # filler units that MUST be emitted before att(qc) starts: proj(qc) is
    # the first 3*qc entries of the original list; track consumed count.
    consumed = [0]

    def emit_filler(n, force=False):
        for _ in range(n):
            if filler and (force or filler[0][0] <= step[0]):
                filler.pop(0)[1]()
                consumed[0] += 1

    pending = [None]

    def pv_step(po_, vt_, lt_, sb_, q0, p_, st, sp):
        for hloc in range(2):
            h = 2 * p_ + hloc
            nc.tensor.matmul(
                po_[:, 512 * hloc + q0:512 * hloc + qw],
                lhsT=vt_[:, lt_, 128 * h:128 * h + 128],
                rhs=sb_[:, 512 * hloc + q0:512 * hloc + qw],
                start=st, stop=sp,
                skip_group_check=True)

    def flush_pv():
        if pending[0] is not None:
            pending[0]()
            pending[0] = None

    for qc in range(n_qch):
        qs = qc * qw
        ktmax = (qs + qw) // 128
        po = {}
        for pair in range(2):
            po[pair] = pop.tile([128, 1024], F32, tag="po",
                                name=f"acc_{pair}_{qc}")
        for kt in range(ktmax):
            off = kt * 128 - qs
            qlo = max(0, off)
            kc, ko = kt // 4, (kt % 4) * 128
            for pair in range(2):
                pt_ps = bigp.tile([128, 1024], F32, tag="big",
                                  name=f"pt_{pair}_{qc}_{kt}")
                if not filler:
                    # HAM keeper: harmless matmuls; scores' start=True
                    # re-clears [qlo:qw], and the junk outside is never read
                    nc.tensor.matmul(
                        pt_ps[:, 0:512], lhsT=warm[:, 0:128], rhs=warm[:],
                        start=True, stop=True, skip_group_check=True)
                for hloc in range(2):
                    nc.tensor.matmul(
                        pt_ps[:, 512 * hloc + qlo:512 * hloc + qw],
                        lhsT=kt_c[kc][64 * hloc:64 * hloc + 64, pair,
                                      ko:ko + 128],
                        rhs=qt_c[qc][64 * hloc:64 * hloc + 64, pair,
                                     qlo:qw],
                        start=True, stop=(off < 0),
                        tile_position=(64 * hloc, 0),
                        skip_group_check=True)
                    if off >= 0:
                        # causal mask: accumulate -240*[q<k] onto the
                        # 128-wide diagonal block; exp then yields ~0 there
                        nc.tensor.matmul(
                            pt_ps[:, 512 * hloc + qlo:512 * hloc + qlo + 128],
                            lhsT=ident[:],
                            rhs=negt[:],
                            start=False, stop=True,
                            skip_group_check=True)
                pt_sb = ptp.tile([128, 1024], BF16, tag="p",
                                 name=f"ptsb_{pair}_{qc}_{kt}")
                pv_ps = pt_ps[:, :].rearrange("p (h x) -> p h x", h=2)
                pv_sb = pt_sb[:, :].rearrange("p (h x) -> p h x", h=2)
                nc.scalar.activation(pv_sb[:, :, qlo:qw], pv_ps[:, :, qlo:qw],
                                     mybir.ActivationFunctionType.Exp,
                                     scale=scale)
                # PV lags one (kt, pair) step: by now its exp has finished
                # during this step's scores burst, so the PE never drains
                if pending[0] is not None:
                    pending[0]()
                pending[0] = (lambda po_=po[pair], vt_=vt_c[kc], lt_=kt % 4,
                              sb_=pt_sb, q0=qlo, p_=pair,
                              st=(kt == 0), sp=(kt == ktmax - 1):
                              pv_step(po_, vt_, lt_, sb_, q0, p_, st, sp))
                step[0] += 1
                emit_filler(1)
        flush_pv()
        for pair in range(2):
            normalize(qc, pair, po[pair])
        if qc + 1 < n_qch:
            drain = 3 * (qc + 1) - consumed[0]
            emit_filler(max(0, drain), force=True)
        # outproj(qc) units become poppable only once the normalize chain
        # producing at(:, :, chunk qc) has had ~4 steps to complete
        for lt in range(4):
            filler.append((step[0] + 4,
                           lambda qc=qc, lt=lt: outproj_unit(qc, lt)))
    emit_filler(len(filler), force=True)


def build_nc(L=2048):
    """Build + compile the per-core Bass program (same NEFF on all 8 cores)."""
    assert L % 512 == 0
    nc = bacc.Bacc("TRN2", target_bir_lowering=False, debug=False,
                   num_devices=N_CORES)
    with tile.TileContext(nc) as tc:
        with tc.tile_pool(name="persist", bufs=1) as pp, \
             tc.tile_pool(name="ropet", bufs=6) as rtp, \
             tc.tile_pool(name="ptp", bufs=3) as ptp, \
             tc.tile_pool(name="rinvp", bufs=2) as rip, \
             tc.tile_pool(name="ostg", bufs=3) as osp, \
             tc.tile_pool(name="bigps", bufs=2, space="PSUM") as bigp, \
             tc.tile_pool(name="pops", bufs=2, space="PSUM") as pop, \
             tc.tile_pool(name="riscr", bufs=4, space="DRAM") as scrp:
            _body(nc, tc, L, pp, rtp, ptp, rip, osp, bigp, pop, scrp)
    nc.compile()
    return nc


_NC_CACHE = {}


def _get_nc(L):
    if L not in _NC_CACHE:
        _NC_CACHE[L] = build_nc(L)
    return _NC_CACHE[L]


def make_inputs(x, token_positions, Wq, Wk, Wv, Wo):
    """Host-side shard/layout prep -> list of 8 per-core input dicts."""
    B, L, _ = x.shape
    pos = np.asarray(token_positions).astype(np.float64)
    S = ROPE_THETA ** (-2.0 / D_HEAD)
    thetas = S ** np.arange(HALF, dtype=np.float64)
    ang = pos[:, None] * thetas[None, :]          # [L, 32]
    cosL = np.cos(ang).T                          # [32, L]
    sinL = np.sin(ang).T
    # per-channel tables on the natural (head, dim) layout:
    # row p (within a 64-row head block): pair i = (p%64)//2
    # cosb[p] = cos(theta_i * pos); ssin[p] = -sin if dim even else +sin
    cosb = np.empty((128, L), dtype=np.float64)
    ssin = np.empty((128, L), dtype=np.float64)
    for p in range(128):
        i = (p % 64) // 2
        cosb[p] = cosL[i]
        ssin[p] = -sinL[i] if (p % 2 == 0) else sinL[i]
    cosb = cosb.astype(NPBF16)
    ssin = ssin.astype(NPBF16)

    r = np.arange(128)[:, None]
    col = np.arange(128)[None, :]
    ident = np.eye(128, dtype=NPBF16)
    negt = np.where(col < r, -240.0, 0.0).astype(NPBF16)  # pre-exp mask bias

    xts = [np.ascontiguousarray(x[b].astype(NPBF16).T) for b in range(B)]
    in_maps = []
    shard_cache = {}
    for core in range(N_CORES):
        b, hg = core // 4, core % 4
        if hg not in shard_cache:
            rows = slice(hg * 256, hg * 256 + 256)
            shard_cache[hg] = {
                "wqt": np.ascontiguousarray(Wq[rows].astype(NPBF16).T),
                "wkt": np.ascontiguousarray(Wk[rows].astype(NPBF16).T),
                "wvt": np.ascontiguousarray(Wv[rows].astype(NPBF16).T),
                "wot": np.ascontiguousarray(Wo[:, rows].astype(NPBF16).T),
            }
        m = dict(shard_cache[hg])
        m["xt"] = xts[b]
        m["cosb"] = cosb
        m["ssin"] = ssin
        m["ident"] = ident
        m["negt"] = negt
        in_maps.append(m)
    return in_maps


def kernel(x, token_positions, Wq, Wk, Wv, Wo):
    x = np.asarray(x); Wq = np.asarray(Wq); Wk = np.asarray(Wk)
    Wv = np.asarray(Wv); Wo = np.asarray(Wo)
    B, L, _ = x.shape
    nc = _get_nc(L)
    in_maps = make_inputs(x, token_positions, Wq, Wk, Wv, Wo)
    res = run_bass_kernel_spmd(nc, in_maps, core_ids=list(range(N_CORES)))
    out = np.zeros((B, L, D_MODEL), dtype=np.float32)
    for core in range(N_CORES):
        out[core // 4] += res.results[core]["out"].astype(np.float32)
    return out


# revision 48
# speedup vs baseline: 1.5510x; 1.2522x over previous
"""Multi-head self-attention (RoPE, causal) Trainium2 Bass kernel, 8 NeuronCores.

Sharding: data-parallel over batch (B=2) x tensor-parallel over heads
(16 heads -> 4 groups of 4). Core c handles batch b=c//4, heads 4*(c%4)..4*(c%4)+3.
Each core computes its 4 heads' attention plus a partial output projection;
the host sums the 4 partial outputs per batch element.

Device-side layout (per core):
  x^T [1024d, L] bf16 (host pre-transposed). Q^T/K^T [256c, L] = W_slice @ x^T.
  RoPE applied in-place on the [channel, L] layout with a partition pair-swap
  (DVE stream_shuffle) + cos / signed-sin tables: 4 DVE ops per tile.
  Attention in transposed [k, q] layout: T = K^T.T @ Q^T (K=64 contraction per
  head), P^T = exp(T/8) (no max subtraction; scores are O(1) by construction),
  O'^T and softmax denominators (ones-matmul) accumulated in PSUM over k tiles.
  Causal: above-diagonal k-tiles skipped entirely, diagonal ones masked.
  Row-group-tiled matmuls must land in distinct PSUM banks, and every open
  PSUM accumulation group owns its whole (partition-range x bank) zero-region.
  Output projection per q chunk from A^T tiles (stationary) x Wo^T slices;
  host sums the 4 partial projections per batch element.
"""
import sys, math

sys.path.insert(0, "/opt/trn_rl_repo")

import numpy as np
import ml_dtypes

import concourse.bacc as bacc
import concourse.bass as bass
import concourse.mybir as mybir
import concourse.tile as tile
from concourse.bass_utils import run_bass_kernel_spmd

BF16 = mybir.dt.bfloat16
F32 = mybir.dt.float32
NPBF16 = ml_dtypes.bfloat16

D_MODEL = 1024
D_HEAD = 64
HALF = D_HEAD // 2
ROPE_THETA = 10000.0
N_CORES = 8
C = 256  # channels per core (4 heads x 64)
SWAP32 = [i ^ 1 for i in range(32)]


def _body(nc, tc, L, pp, rtp, ptp, rip, osp):
    n_lt = L // 128
    n_qk = max(1, L // 512)
    qkw = min(512, L)
    qw = min(512, L)
    n_qch = L // qw
    n_msk = qw // 128

    xt_d = nc.dram_tensor("xt", [D_MODEL, L], BF16, kind="ExternalInput").ap()
    wq_d = nc.dram_tensor("wqt", [D_MODEL, C], BF16, kind="ExternalInput").ap()
    wk_d = nc.dram_tensor("wkt", [D_MODEL, C], BF16, kind="ExternalInput").ap()
    wv_d = nc.dram_tensor("wvt", [D_MODEL, C], BF16, kind="ExternalInput").ap()
    wo_d = nc.dram_tensor("wot", [C, D_MODEL], BF16, kind="ExternalInput").ap()
    cos_d = nc.dram_tensor("cosb", [128, L], BF16, kind="ExternalInput").ap()
    sin_d = nc.dram_tensor("ssin", [128, L], BF16, kind="ExternalInput").ap()
    mk_d = nc.dram_tensor("masks", [128, 128], BF16,
                          kind="ExternalInput").ap()
    out_d = nc.dram_tensor("out", [L, D_MODEL], BF16, kind="ExternalOutput").ap()

    # ---- persistent SBUF tensors
    wq = pp.tile([128, 8, C], BF16)
    wk = pp.tile([128, 8, C], BF16)
    wv = pp.tile([128, 8, C], BF16)
    wo = pp.tile([128, 2, D_MODEL], BF16)
    cs = pp.tile([128, L], BF16)
    sn = pp.tile([128, L], BF16)
    mks = pp.tile([128, 128], BF16)
    ones = pp.tile([128, 64], BF16)
    warm = pp.tile([128, 512], BF16)
    n_ch = max(1, L // 512)
    chw = min(512, L)
    qt_c = [pp.tile([128, 2, chw], BF16, name=f"qt{i}") for i in range(n_ch)]
    kt_c = [pp.tile([128, 2, chw], BF16, name=f"ktc{i}") for i in range(n_ch)]
    vt_c = [pp.tile([128, chw // 128, C + 4], BF16, name=f"vt{i}")
            for i in range(n_ch)]
    at = pp.tile([128, 2, L], BF16)
    xts = [pp.tile([128, L], BF16, name=f"xt{i}") for i in range(8)]

    # ---- loads (small tensors first; x^T split per d-tile for pipelining)
    nc.sync.dma_start(out=wq[:], in_=wq_d.rearrange("(a p) c -> p a c", p=128))
    nc.sync.dma_start(out=xts[0][:], in_=xt_d[0:128, :])
    nc.sync.dma_start(out=xts[1][:], in_=xt_d[128:256, :])
    nc.sync.dma_start(out=wk[:], in_=wk_d.rearrange("(a p) c -> p a c", p=128))
    for i in range(2, 8):
        nc.sync.dma_start(out=xts[i][:], in_=xt_d[i * 128:(i + 1) * 128, :])
    nc.sync.dma_start(out=cs[:], in_=cos_d)
    nc.sync.dma_start(out=sn[:], in_=sin_d)
    nc.sync.dma_start(out=wv[:], in_=wv_d.rearrange("(a p) c -> p a c", p=128))
    nc.sync.dma_start(out=wo[:], in_=wo_d.rearrange("(a p) e -> p a e", p=128))
    nc.sync.dma_start(out=mks[:], in_=mk_d)
    nc.gpsimd.memset(warm[:], 0.0)
    nc.gpsimd.memset(ones[:], 1.0)
    for i in range(len(vt_c)):
        ov = vt_c[i][:, :, :].rearrange("p l (h x) -> p l h x", x=65)
        nc.gpsimd.memset(ov[:, :, :, 64], 1.0)

    # ---- Q^T / K^T projection + RoPE (shuffle pair-swap + cos/signed-sin)
    with tc.tile_pool(name="qk_ps", bufs=6, space="PSUM") as qkps, \
         tc.tile_pool(name="v_ps", bufs=2, space="PSUM") as vps:
        wps = qkps.tile([128, qkw], F32, tag="qkps", name="warm_ps")
        for r in range(16):
            nc.tensor.matmul(wps[:], lhsT=warm[:, 0:128], rhs=warm[:],
                             start=(r == 0), stop=(r == 15))
        for qc in range(n_qk):
            ls = qc * qkw
            ps = {}
            for nm, w in (("q", wq), ("k", wk)):
                for ct in (0, 1):
                    p = qkps.tile([128, qkw], F32, tag="qkps",
                                  name=f"ps_{nm}{ct}_{qc}")
                    for dt_ in range(8):
                        nc.tensor.matmul(
                            p[:],
                            lhsT=w[:, dt_, ct * 128:ct * 128 + 128],
                            rhs=xts[dt_][:, ls:ls + qkw],
                            start=(dt_ == 0), stop=(dt_ == 7))
                    ps[(nm, ct)] = p
            for nm, dstc in (("q", qt_c), ("k", kt_c)):
                dst = dstc[qc]
                for ct in (0, 1):
                    p = ps[(nm, ct)]
                    sh = rtp.tile([128, qkw], F32, tag="t",
                                  name=f"sh_{nm}{ct}{qc}")
                    t1 = rtp.tile([128, qkw], F32, tag="t",
                                  name=f"t1_{nm}{ct}{qc}")
                    t2 = rtp.tile([128, qkw], F32, tag="t",
                                  name=f"t2_{nm}{ct}{qc}")
                    nc.vector.stream_shuffle(sh[:], p[:], SWAP32)
                    nc.vector.tensor_mul(t1[:], p[:], cs[:, ls:ls + qkw])
                    nc.gpsimd.tensor_mul(t2[:], sh[:], sn[:, ls:ls + qkw])
                    nc.gpsimd.tensor_add(dst[:, ct, :], t1[:], t2[:])
            # V projection for this chunk's L tiles (keeps attention startable)
            for lt in range(ls // 128, (ls + qkw) // 128):
                pv = vps.tile([128, C], F32, tag="vps", name=f"pv_{lt}")
                for dt_ in range(8):
                    nc.tensor.matmul(
                        pv[:],
                        lhsT=xts[dt_][:, lt * 128:lt * 128 + 128],
                        rhs=wv[:, dt_, :],
                        start=(dt_ == 0), stop=(dt_ == 7))
                ov = vt_c[lt // 4][:, lt % 4, :].rearrange(
                    "p (h x) -> p h x", x=65)[:, :, 0:64]
                nc.scalar.copy(ov, pv[:].rearrange("p (h x) -> p h x", x=64))


    # ---- attention + interleaved output projection, per 512-wide q chunk
    scale = 1.0 / math.sqrt(D_HEAD)
    with tc.tile_pool(name="att_ps", bufs=2, space="PSUM") as atps, \
         tc.tile_pool(name="o_ps", bufs=2, space="PSUM") as ops_, \
         tc.tile_pool(name="riscr_p", bufs=4, space="DRAM") as scrp:
        for pair in range(2):
            for qc in range(n_qch):
                qs = qc * qw
                ktmax = (qs + qw) // 128
                po = ops_.tile([128, 1024], F32, tag="o", name=f"po_{pair}_{qc}")
                for kt in range(ktmax):
                    off = kt * 128 - qs
                    qlo = max(0, off)      # only q >= k contributes
                    kc, ko = kt // 4, (kt % 4) * 128
                    pt_ps = atps.tile([128, 1024], F32, tag="tps",
                                      name=f"pt_{pair}_{qc}_{kt}")
                    for hloc in range(2):
                        nc.tensor.matmul(
                            pt_ps[:, 512 * hloc + qlo:512 * hloc + qw],
                            lhsT=kt_c[kc][64 * hloc:64 * hloc + 64, pair,
                                          ko:ko + 128],
                            rhs=qt_c[qc][64 * hloc:64 * hloc + 64, pair,
                                         qlo:qw],
                            start=True, stop=True,
                            tile_position=(64 * hloc, 0),
                            skip_group_check=True)
                    pt_sb = ptp.tile([128, 1024], BF16, tag="p",
                                     name=f"ptsb_{pair}_{qc}_{kt}")
                    pv_ps = pt_ps[:, :].rearrange("p (h x) -> p h x", h=2)
                    pv_sb = pt_sb[:, :].rearrange("p (h x) -> p h x", h=2)
                    nc.scalar.activation(pv_sb[:, :, qlo:qw],
                                         pv_ps[:, :, qlo:qw],
                                         mybir.ActivationFunctionType.Exp,
                                         scale=scale)
                    if off >= 0:
                        # mask the 128-wide diagonal block (tril); rest kept
                        for hloc in range(2):
                            nc.vector.tensor_mul(
                                pt_sb[:, 512 * hloc + qlo:512 * hloc + qlo + 128],
                                pt_sb[:, 512 * hloc + qlo:512 * hloc + qlo + 128],
                                mks[:, 0:128])
                    for hloc in range(2):
                        h = 2 * pair + hloc
                        # lhsT [V_h | 1]: row 64 of the output accumulates the
                        # softmax denominators for free
                        nc.tensor.matmul(
                            po[0:65, 512 * hloc + qlo:512 * hloc + qw],
                            lhsT=vt_c[kc][:, kt % 4, 65 * h:65 * h + 65],
                            rhs=pt_sb[:, 512 * hloc + qlo:512 * hloc + qw],
                            start=(kt == 0), stop=(kt == ktmax - 1),
                            skip_group_check=True)
                rrow = rip.tile([1, 1024], F32, tag="ri",
                                name=f"rr_{pair}_{qc}")
                if qw == 512:
                    nc.vector.tensor_copy(rrow[:], po[64:65, :])
                else:
                    for hloc in range(2):
                        nc.vector.tensor_copy(
                            rrow[:, qw * hloc:qw * hloc + qw],
                            po[64:65, 512 * hloc:512 * hloc + qw])
                scrt = scrp.tile([1, 1024], F32, tag="scr",
                                 name=f"scr_{pair}_{qc}")
                scr = scrt[:, 0:2 * qw]
                nc.sync.dma_start(out=scr, in_=rrow[:, 0:2 * qw])
                pb = rip.tile([64, 1024], F32, tag="pb",
                              name=f"pb_{pair}_{qc}")
                nc.sync.dma_start(out=pb[:, 0:2 * qw],
                                  in_=scr.partition_broadcast(64))
                pbi = rip.tile([64, 1024], F32, tag="pbi",
                               name=f"pbi_{pair}_{qc}")
                nc.vector.reciprocal_approx_fast(out=pbi[:, 0:2 * qw],
                                                 in_=pb[:, 0:2 * qw])
                tm = rip.tile([64, 1024], BF16, tag="tm",
                              name=f"tm_{pair}_{qc}")
                if qw == 512:
                    nc.vector.tensor_mul(tm[:], po[0:64, :], pbi[:])
                else:
                    for hloc in range(2):
                        nc.vector.tensor_mul(
                            tm[:, 512 * hloc:512 * hloc + qw],
                            po[0:64, 512 * hloc:512 * hloc + qw],
                            pbi[:, qw * hloc:qw * hloc + qw])
                nc.vector.tensor_copy(at[0:64, pair, qs:qs + qw],
                                      tm[:, 0:qw])
                nc.sync.dma_start(out=at[64:128, pair, qs:qs + qw],
                                  in_=tm[:, 512:512 + qw])
    # ---- output projection (separate phase; attention keeps 8 psum banks)
    with tc.tile_pool(name="out_ps", bufs=2, space="PSUM") as outps:
        for qtl in range(n_lt):
            pout = outps.tile([128, 1024], F32, tag="outps",
                              name=f"pout_{qtl}")
            for ct in range(2):
                for eh in range(2):
                    nc.tensor.matmul(
                        pout[:, eh * 512:eh * 512 + 512],
                        lhsT=at[:, ct, qtl * 128:qtl * 128 + 128],
                        rhs=wo[:, ct, eh * 512:eh * 512 + 512],
                        start=(ct == 0), stop=(ct == 1),
                        skip_group_check=True)
            stg = osp.tile([128, 1024], BF16, tag="stg", name=f"stg_{qtl}")
            nc.vector.tensor_copy(stg[:, 0:512], pout[:, 0:512])
            nc.scalar.copy(stg[:, 512:1024], pout[:, 512:1024])
            nc.sync.dma_start(out=out_d[qtl * 128:qtl * 128 + 128, :],
                              in_=stg[:])


def build_nc(L=2048):
    """Build + compile the per-core Bass program (same NEFF on all 8 cores)."""
    assert L % 256 == 0
    nc = bacc.Bacc("TRN2", target_bir_lowering=False, debug=False,
                   num_devices=N_CORES)
    with tile.TileContext(nc) as tc:
        with tc.tile_pool(name="persist", bufs=1) as pp, \
             tc.tile_pool(name="ropet", bufs=6) as rtp, \
             tc.tile_pool(name="ptp", bufs=3) as ptp, \
             tc.tile_pool(name="rinvp", bufs=2) as rip, \
             tc.tile_pool(name="ostg", bufs=3) as osp:
            _body(nc, tc, L, pp, rtp, ptp, rip, osp)
    nc.compile()
    return nc


_NC_CACHE = {}


def _get_nc(L):
    if L not in _NC_CACHE:
        _NC_CACHE[L] = build_nc(L)
    return _NC_CACHE[L]


def make_inputs(x, token_positions, Wq, Wk, Wv, Wo):
    """Host-side shard/layout prep -> list of 8 per-core input dicts."""
    B, L, _ = x.shape
    pos = np.asarray(token_positions).astype(np.float64)
    S = ROPE_THETA ** (-2.0 / D_HEAD)
    thetas = S ** np.arange(HALF, dtype=np.float64)
    ang = pos[:, None] * thetas[None, :]          # [L, 32]
    cosL = np.cos(ang).T                          # [32, L]
    sinL = np.sin(ang).T
    # per-channel tables on the natural (head, dim) layout:
    # row p (within a 64-row head block): pair i = (p%64)//2
    # cosb[p] = cos(theta_i * pos); ssin[p] = -sin if dim even else +sin
    cosb = np.empty((128, L), dtype=np.float64)
    ssin = np.empty((128, L), dtype=np.float64)
    for p in range(128):
        i = (p % 64) // 2
        cosb[p] = cosL[i]
        ssin[p] = -sinL[i] if (p % 2 == 0) else sinL[i]
    cosb = cosb.astype(NPBF16)
    ssin = ssin.astype(NPBF16)

    r = np.arange(128)[:, None]
    col = np.arange(128)[None, :]
    masks = (col >= r).astype(NPBF16)  # [128, 128] tril(keep q>=k)

    xts = [np.ascontiguousarray(x[b].astype(NPBF16).T) for b in range(B)]
    in_maps = []
    shard_cache = {}
    for core in range(N_CORES):
        b, hg = core // 4, core % 4
        if hg not in shard_cache:
            rows = slice(hg * 256, hg * 256 + 256)
            shard_cache[hg] = {
                "wqt": np.ascontiguousarray(Wq[rows].astype(NPBF16).T),
                "wkt": np.ascontiguousarray(Wk[rows].astype(NPBF16).T),
                "wvt": np.ascontiguousarray(Wv[rows].astype(NPBF16).T),
                "wot": np.ascontiguousarray(Wo[:, rows].astype(NPBF16).T),
            }
        m = dict(shard_cache[hg])
        m["xt"] = xts[b]
        m["cosb"] = cosb
        m["ssin"] = ssin
        m["masks"] = masks
        in_maps.append(m)
    return in_maps


def kernel(x, token_positions, Wq, Wk, Wv, Wo):
    x = np.asarray(x); Wq = np.asarray(Wq); Wk = np.asarray(Wk)
    Wv = np.asarray(Wv); Wo = np.asarray(Wo)
    B, L, _ = x.shape
    nc = _get_nc(L)
    in_maps = make_inputs(x, token_positions, Wq, Wk, Wv, Wo)
    res = run_bass_kernel_spmd(nc, in_maps, core_ids=list(range(N_CORES)))
    out = np.zeros((B, L, D_MODEL), dtype=np.float32)
    for core in range(N_CORES):
        out[core // 4] += res.results[core]["out"].astype(np.float32)
    return out

